# revision 1
# baseline (speedup 1.0000x reference)
"""Correlation1D Trainium2 Bass kernel.

out[b, d, h, w] = (1/C) * sum_c in1[b, c, h, w] * in2pad[b, c, h, w + d]
  B=8, C=256, H=96, W=192, PAD=40, D=81 displacement channels.

Strategy (data-parallel over batch, 1 sample per NeuronCore):
  For each h row and each w-chunk of 96, compute a Gram band
      G[w, w'] = sum_c in1[c, w] * in2pad[c, w']      (PE matmuls, k=c)
  for w' in [ck*96, ck*96 + 176).  The needed outputs are the 81
  diagonals O[d, w] = G[w, w + d].  Diagonals can't be extracted by any
  on-chip AP (partition/free strides are independent), so the band is
  written to DRAM scratch where a *flat* strided access pattern CAN walk
  diagonals: a gather DMA with partition stride (row_stride + 1) yields
  T[w, d] = G[w, w + d].  A PE transpose then gives O[d, w] tiles which
  are written out in the final [d, h, w] layout.

  Matmuls run in float32r (TF32-like, ~1.4e-4 rel err, 4x faster than
  fp32 at N>=256 — hence the fully padded 272-wide rhs); inputs are
  rounded to fp32r for free by SWDGE casting DMA loads.  The band is
  stored fp16 (values are pre-scaled by 1/C, ~5e-4 rel err) which halves
  scratch traffic.
"""

import os

import numpy as np

import bass_rust as _br
import concourse.bass as bass
import concourse.tile as tile
from concourse import bacc, mybir
from concourse.bass_utils import run_bass_kernel_spmd

# Problem constants (hardcoded per harness contract)
B = 8
C = 256
H = 96
W = 192
PAD = 40
D = 2 * PAD + 1  # 81
W2 = W + 2 * PAD  # 272 padded width
CH = 2  # c is split into CH partition-halves of 128
CP = C // CH  # 128
CHUNK = 96  # w-chunk (Gram output partition dim)
NCK = W // CHUNK  # 2
BANDW = CHUNK + D - 1  # 176  (w' window width per chunk)

# Tunables (env-overridable for experiments)
HB = int(os.environ.get("CORR_HB", "4"))  # h rows per batch
NB = H // HB
BAND_DT_S = os.environ.get("CORR_BAND_DT", "fp16")  # fp32 | fp16 | bf16
MM_DT_S = os.environ.get("CORR_MM", "fp32r")  # fp32 | fp32r
IN_BUFS = int(os.environ.get("CORR_IN_BUFS", "3"))
G_BUFS = int(os.environ.get("CORR_G_BUFS", "4"))

_DT = {
    "fp32": mybir.dt.float32,
    "fp16": mybir.dt.float16,
    "bf16": mybir.dt.bfloat16,
    "fp32r": mybir.dt.float32r,
}


def _build(reps=1):
    band_dt = _DT[BAND_DT_S]
    mm_dt = _DT[MM_DT_S]
    f32 = mybir.dt.float32

    nc = bacc.Bacc("TRN2")

    in1 = nc.dram_tensor("input1", [C, H, W], f32, kind="ExternalInput")
    in2 = nc.dram_tensor("input2", [C, H, W], f32, kind="ExternalInput")
    out = nc.dram_tensor("out", [D, H, W], f32, kind="ExternalOutput")
    scratch = nc.dram_tensor("scratch", [NCK, CHUNK, H, BANDW], band_dt)

    # [c, h, w] -> [p, a, h*w] so the in1 load is one 3-dim DMA
    in1_r = in1.ap().rearrange("(a p) h w -> p a (h w)", p=CP)
    in2_r = in2.ap().rearrange("(a p) h w -> p a h w", p=CP)
    out_ap = out.ap()
    scr_ap = scratch.ap()

    # casting loads (fp32 -> fp32r rounding) need SWDGE; plain fp32 can
    # use the faster HWDGE path
    load_eng = nc.gpsimd if MM_DT_S == "fp32r" else nc.sync

    with tile.TileContext(nc) as tc:
        with (
            tc.tile_pool(name="singles", bufs=1) as singles,
            tc.tile_pool(name="loads", bufs=IN_BUFS) as loads,
            tc.tile_pool(name="bands", bufs=2) as bands,
            tc.tile_pool(name="gats", bufs=2) as gats,
            tc.tile_pool(name="outs", bufs=2) as outs,
            tc.tile_pool(name="psg", bufs=G_BUFS, space="PSUM") as psg,
            tc.tile_pool(name="pso", bufs=2, space="PSUM") as pso,
        ):
            # identity for PE transposes
            ident = singles.tile([CHUNK, CHUNK], band_dt)
            from concourse.masks import make_identity

            make_identity(nc, ident[:])

            for _rep in range(reps):
              for ib in range(NB):
                h0 = ib * HB

                in1_t = loads.tile([CP, CH, HB, W], mm_dt)
                load_eng.dma_start(
                    out=in1_t[:].rearrange("p a h w -> p a (h w)"),
                    in_=in1_r[:, :, h0 * W : (h0 + HB) * W],
                )
                # in2 is zero-padded to 272 cols so the matmul free dim is
                # >=256 (fp32r full-rate threshold)
                in2_t = loads.tile([CP, CH, HB, W2], mm_dt)
                # memset doesn't accept fp32r — zero through an f32 view
                # (zero bits are dtype-invariant)
                nc.gpsimd.memset(in2_t[:, :, :, 0:PAD].bitcast(f32), 0.0)
                nc.gpsimd.memset(in2_t[:, :, :, PAD + W : W2].bitcast(f32), 0.0)
                for a in range(CH):
                    load_eng.dma_start(
                        out=in2_t[:, a, :, PAD : PAD + W],
                        in_=in2_r[:, a, h0 : h0 + HB, :],
                    )

                band_ts = [
                    bands.tile(
                        [CHUNK, HB, BANDW], band_dt,
                        name=f"band{ck}_{_rep}_{ib}", tag=f"band{ck}",
                    )
                    for ck in range(NCK)
                ]

                for hl in range(HB):
                    for ck in range(NCK):
                        g = psg.tile([CHUNK, W2], f32)
                        for a in range(CH):
                            nc.tensor.matmul(
                                g[:],
                                in1_t[:, a, hl, ck * CHUNK : (ck + 1) * CHUNK],
                                in2_t[:, a, hl, :],
                                start=(a == 0),
                                stop=(a == CH - 1),
                            )
                        # extract band + 1/C scale (+ cast to band_dt)
                        nc.scalar.mul(
                            out=band_ts[ck][:, hl, :],
                            in_=g[:, ck * CHUNK : ck * CHUNK + BANDW],
                            mul=1.0 / C,
                        )

                band_dmas = []
                for ck in range(NCK):
                    di = nc.sync.dma_start(
                        out=scr_ap[ck, :, h0 : h0 + HB, :],
                        in_=band_ts[ck][:],
                    )
                    band_dmas.append(di)

                # --- phase 2: skewed gather + transpose + writeout ---
                gat_ts = []
                for ck in range(NCK):
                    gat = gats.tile(
                        [CHUNK, HB, D], band_dt,
                        name=f"gat{ck}_{_rep}_{ib}", tag=f"gat{ck}",
                    )
                    skew = bass.AP(
                        tensor=scr_ap.tensor,
                        offset=ck * (CHUNK * H * BANDW) + h0 * BANDW,
                        ap=[[H * BANDW + 1, CHUNK], [BANDW, HB], [1, D]],
                    )
                    gi = nc.sync.dma_start(out=gat[:], in_=skew)
                    # Explicit RAW edges through DRAM scratch (belt & braces
                    # in case AP-overlap detection misses the skewed stride).
                    _br.add_dep_helper(
                        gi.ins, band_dmas[ck].ins, reason="scratch RAW"
                    )
                    gat_ts.append(gat)

                out_t = outs.tile([D, HB, W], f32)
                for hl in range(HB):
                    po = pso.tile([D, W], band_dt)
                    for ck in range(NCK):
                        nc.tensor.transpose(
                            out=po[:, ck * CHUNK : (ck + 1) * CHUNK],
                            in_=gat_ts[ck][:, hl, :],
                            identity=ident[:],
                        )
                    nc.vector.tensor_copy(out=out_t[:, hl, :], in_=po[:])
                nc.sync.dma_start(out=out_ap[:, h0 : h0 + HB, :], in_=out_t[:])

    nc.compile()
    return nc


_NC_CACHE = None


def run(input1, input2, trace=False, **spmd_kwargs):
    """Run on 8 NeuronCores; returns (out [B,D,H,W] fp32, BassKernelResults)."""
    global _NC_CACHE
    if _NC_CACHE is None:
        _NC_CACHE = _build()
    nc = _NC_CACHE

    input1 = np.ascontiguousarray(np.asarray(input1), dtype=np.float32)
    input2 = np.ascontiguousarray(np.asarray(input2), dtype=np.float32)
    assert input1.shape == (B, C, H, W) and input2.shape == (B, C, H, W)

    in_maps = [
        {"input1": input1[b], "input2": input2[b]} for b in range(B)
    ]
    res = run_bass_kernel_spmd(
        nc, in_maps, core_ids=list(range(B)), trace=trace, **spmd_kwargs
    )
    out = np.stack([res.results[b]["out"] for b in range(B)], axis=0)
    return out, res


def kernel(input1, input2):
    out, _ = run(input1, input2)
    return out



# revision 2
# speedup vs baseline: 1.7678x; 1.7678x over previous
"""Correlation1D Trainium2 Bass kernel.

out[b, d, h, w] = (1/C) * sum_c in1[b, c, h, w] * in2pad[b, c, h, w + d]
  B=8, C=256, H=96, W=192, PAD=40, D=81 displacement channels.

Strategy (data-parallel over batch, 1 sample per NeuronCore):
  For each h row and each w-chunk of 96, compute a Gram band
      G[w, w'] = sum_c in1[c, w] * in2pad[c, w']      (PE matmuls, k=c)
  for w' in [ck*96, ck*96 + 176).  The needed outputs are the 81
  diagonals O[d, w] = G[w, w + d].  Diagonal extraction is a
  per-partition-offset move: partition w needs band columns [w, w+81).
  GPSIMD's local_scatter supports per-partition independent indices
  (dst[p, idx[p,i]] = data[p,i], -1 skips), so a static int16 index
  tile (fed as an extra kernel input) extracts all diagonals on-chip —
  no DRAM scratch round-trip.  HBM traffic is just inputs + output
  (43.7 MB/core vs 53.3 MB with the old DRAM-scratch skew-gather).

  A PE transpose then turns T[w, d] tiles into O[d, w] tiles which are
  written out in the final [d, h, w] layout.

  Matmuls run in float32r (TF32-like, ~1.4e-4 rel err, 4x faster than
  fp32 at N>=256 — hence the fully padded 272-wide rhs); inputs are
  rounded to fp32r for free by SWDGE casting DMA loads.  The band is
  fp16 in SBUF (values are pre-scaled by 1/C, ~5e-4 rel err; 2-byte
  dtype is also what local_scatter requires).
"""

import os

import numpy as np

import concourse.bass as bass
import concourse.tile as tile
from concourse import bacc, mybir
from concourse.bass_utils import run_bass_kernel_spmd

# Problem constants (hardcoded per harness contract)
B = 8
C = 256
H = 96
W = 192
PAD = 40
D = 2 * PAD + 1  # 81
W2 = W + 2 * PAD  # 272 padded width
CH = 2  # c is split into CH partition-halves of 128
CP = C // CH  # 128
CHUNK = 96  # w-chunk (Gram output partition dim)
NCK = W // CHUNK  # 2
BANDW = CHUNK + D - 1  # 176  (w' window width per chunk)
DE = D + 1  # 82: even-sized diagonal slot per h row (local_scatter needs %2)

# Tunables (env-overridable for experiments)
HB = int(os.environ.get("CORR_HB", "4"))  # h rows per batch
NB = H // HB
BAND_DT_S = os.environ.get("CORR_BAND_DT", "fp16")  # fp16 | bf16
MM_DT_S = os.environ.get("CORR_MM", "fp32r")  # fp32 | fp32r
IN_BUFS = int(os.environ.get("CORR_IN_BUFS", "3"))
G_BUFS = int(os.environ.get("CORR_G_BUFS", "4"))

_DT = {
    "fp32": mybir.dt.float32,
    "fp16": mybir.dt.float16,
    "bf16": mybir.dt.bfloat16,
    "fp32r": mybir.dt.float32r,
}


def make_diag_idx() -> np.ndarray:
    """Static local_scatter indices: idx[w, hl*BANDW + j] = hl*DE + (j - w)
    when 0 <= j - w < D, else -1 (skipped)."""
    idx = np.full((CHUNK, HB * BANDW), -1, dtype=np.int16)
    w = np.arange(CHUNK)[:, None]
    j = np.arange(BANDW)[None, :]
    d = j - w  # [CHUNK, BANDW]
    valid = (d >= 0) & (d < D)
    for hl in range(HB):
        blk = np.where(valid, hl * DE + d, -1).astype(np.int16)
        idx[:, hl * BANDW : (hl + 1) * BANDW] = blk
    return idx


def _build(reps=1):
    band_dt = _DT[BAND_DT_S]
    mm_dt = _DT[MM_DT_S]
    f32 = mybir.dt.float32
    i16 = mybir.dt.int16

    nc = bacc.Bacc("TRN2")

    in1 = nc.dram_tensor("input1", [C, H, W], f32, kind="ExternalInput")
    in2 = nc.dram_tensor("input2", [C, H, W], f32, kind="ExternalInput")
    didx = nc.dram_tensor("didx", [CHUNK, HB * BANDW], i16, kind="ExternalInput")
    out = nc.dram_tensor("out", [D, H, W], f32, kind="ExternalOutput")

    # [c, h, w] -> [p, a, h*w] so the in1 load is one 3-dim DMA
    in1_r = in1.ap().rearrange("(a p) h w -> p a (h w)", p=CP)
    in2_r = in2.ap().rearrange("(a p) h w -> p a h w", p=CP)
    out_ap = out.ap()

    # casting loads (fp32 -> fp32r rounding) need SWDGE; plain fp32 can
    # use the faster HWDGE path
    load_eng = nc.gpsimd if MM_DT_S == "fp32r" else nc.sync

    with tile.TileContext(nc) as tc:
        with (
            tc.tile_pool(name="singles", bufs=1) as singles,
            tc.tile_pool(name="loads", bufs=IN_BUFS) as loads,
            tc.tile_pool(name="bands", bufs=2) as bands,
            tc.tile_pool(name="gats", bufs=2) as gats,
            tc.tile_pool(name="outs", bufs=2) as outs,
            tc.tile_pool(name="psg", bufs=G_BUFS, space="PSUM") as psg,
            tc.tile_pool(name="pso", bufs=2, space="PSUM") as pso,
        ):
            # identity for PE transposes
            ident = singles.tile([CHUNK, CHUNK], band_dt)
            from concourse.masks import make_identity

            make_identity(nc, ident[:])

            # static per-partition diagonal indices, loaded once
            idx_t = singles.tile([CHUNK, HB * BANDW], i16)
            nc.sync.dma_start(out=idx_t[:], in_=didx.ap())

            for _rep in range(reps):
              for ib in range(NB):
                h0 = ib * HB

                in1_t = loads.tile([CP, CH, HB, W], mm_dt)
                load_eng.dma_start(
                    out=in1_t[:].rearrange("p a h w -> p a (h w)"),
                    in_=in1_r[:, :, h0 * W : (h0 + HB) * W],
                )
                # in2 is zero-padded to 272 cols so the matmul free dim is
                # >=256 (fp32r full-rate threshold)
                in2_t = loads.tile([CP, CH, HB, W2], mm_dt)
                # memset doesn't accept fp32r — zero through an f32 view
                # (zero bits are dtype-invariant)
                nc.vector.memset(in2_t[:, :, :, 0:PAD].bitcast(f32), 0.0)
                nc.vector.memset(in2_t[:, :, :, PAD + W : W2].bitcast(f32), 0.0)
                for a in range(CH):
                    load_eng.dma_start(
                        out=in2_t[:, a, :, PAD : PAD + W],
                        in_=in2_r[:, a, h0 : h0 + HB, :],
                    )

                band_ts = [
                    bands.tile(
                        [CHUNK, HB, BANDW], band_dt,
                        name=f"band{ck}_{_rep}_{ib}", tag=f"band{ck}",
                    )
                    for ck in range(NCK)
                ]

                for hl in range(HB):
                    for ck in range(NCK):
                        g = psg.tile([CHUNK, W2], f32)
                        for a in range(CH):
                            nc.tensor.matmul(
                                g[:],
                                in1_t[:, a, hl, ck * CHUNK : (ck + 1) * CHUNK],
                                in2_t[:, a, hl, :],
                                start=(a == 0),
                                stop=(a == CH - 1),
                            )
                        # extract band + 1/C scale (+ cast to band_dt)
                        nc.scalar.mul(
                            out=band_ts[ck][:, hl, :],
                            in_=g[:, ck * CHUNK : ck * CHUNK + BANDW],
                            mul=1.0 / C,
                        )

                # --- phase 2: on-chip diagonal extraction (local_scatter:
                # dst[w, hl*DE + (j-w)] = band[w, hl*BANDW + j]) ---
                gat_ts = []
                for ck in range(NCK):
                    gat = gats.tile(
                        [CHUNK, HB, DE], band_dt,
                        name=f"gat{ck}_{_rep}_{ib}", tag=f"gat{ck}",
                    )
                    nc.gpsimd.local_scatter(
                        out_ap=gat[:],
                        data_ap=band_ts[ck][:],
                        idxs_ap=idx_t[:],
                        channels=CHUNK,
                        num_elems=HB * DE,
                        num_idxs=HB * BANDW,
                    )
                    gat_ts.append(gat)

                out_t = outs.tile([D, HB, W], f32)
                for hl in range(HB):
                    po = pso.tile([D, W], band_dt)
                    for ck in range(NCK):
                        nc.tensor.transpose(
                            out=po[:, ck * CHUNK : (ck + 1) * CHUNK],
                            in_=gat_ts[ck][:, hl, 0:D],
                            identity=ident[:],
                        )
                    nc.vector.tensor_copy(out=out_t[:, hl, :], in_=po[:])
                nc.sync.dma_start(out=out_ap[:, h0 : h0 + HB, :], in_=out_t[:])

    nc.compile()
    return nc


_NC_CACHE = None


def run(input1, input2, trace=False, **spmd_kwargs):
    """Run on 8 NeuronCores; returns (out [B,D,H,W] fp32, BassKernelResults)."""
    global _NC_CACHE
    if _NC_CACHE is None:
        _NC_CACHE = _build()
    nc = _NC_CACHE

    input1 = np.ascontiguousarray(np.asarray(input1), dtype=np.float32)
    input2 = np.ascontiguousarray(np.asarray(input2), dtype=np.float32)
    assert input1.shape == (B, C, H, W) and input2.shape == (B, C, H, W)

    didx = make_diag_idx()
    in_maps = [
        {"input1": input1[b], "input2": input2[b], "didx": didx}
        for b in range(B)
    ]
    res = run_bass_kernel_spmd(
        nc, in_maps, core_ids=list(range(B)), trace=trace, **spmd_kwargs
    )
    out = np.stack([res.results[b]["out"] for b in range(B)], axis=0)
    return out, res


def kernel(input1, input2):
    out, _ = run(input1, input2)
    return out
